# revision 3
# baseline (speedup 1.0000x reference)
"""Supervised-contrastive loss on 8 Trainium2 NeuronCores — symmetric v2.

Exploits sim-matrix symmetry: only the 136 upper-triangle 512x512 blocks of
the 16x16 block grid are computed (vs all 64 [1024 x 8192] slab blocks in
v1), nearly halving both the PE matmul work and the ScalarE Exp work.

Each core gets 17 blocks as six same-row strips of widths [4,4,4,2,2,1]
blocks (the unique uniform multiset that tiles the triangle's row lengths),
so the single SPMD program is identical across cores; which (i, j) a strip
holds is pure data (host-packed operand layout + host-side unpacking).

Per (strip, m-subtile of 128 anchors): 3 DoubleRow fp8 matmuls per 512-col
block accumulate sim into PSUM [128, W]; one ScalarE Exp writes E = exp(10
sim) as bf16 to SBUF with accum_out giving the row sums. Mirror (column)
sums: the bf16 E tiles are DMAd back to HBM (fully overlapped with the
remaining matmuls) and the HOST does the per-block column sums -- measured
on-device alternatives (GpSimd partition_all_reduce at 3.8us/block, DVE
adds at 1.3us each) are far slower than the PE and would backpressure it.
Class-segment sums stay the v1 small GEMM tm = A @ W.T. Host reconstructs
es[j] from row + mirror sums, subtracts the exact diagonal
exp(10*||z8_i||^2), and finishes the loss in float64.
"""

import numpy as np
import ml_dtypes


def _ensure_ntff_hook():
    """Install the antenv.axon_hooks NTFF profile hook if the image's antenv
    package lacks it (bass_utils imports it unconditionally when BASS_TRACE
    is set; without this shim the run crashes instead of profiling)."""
    import importlib.util
    import sys
    import types

    try:
        if importlib.util.find_spec("antenv.axon_hooks") is not None:
            return
    except (ImportError, ModuleNotFoundError):
        pass
    try:
        import antenv
        from trn_agent_boot.trn_boot import _ntff_profile_via_ctypes
    except ImportError:
        return
    try:
        hook = _ntff_profile_via_ctypes("/opt/axon/libaxon_pjrt.so")
    except Exception:
        hook = None
    m = types.ModuleType("antenv.axon_hooks")
    _state = {"hook": hook}
    m.get_axon_ntff_profile_hook = lambda: _state["hook"]
    m.set_axon_ntff_profile_hook = lambda h: _state.__setitem__("hook", h)
    sys.modules["antenv.axon_hooks"] = m
    antenv.axon_hooks = m


_ensure_ntff_hook()

N = 8192
D = 768
NOP = 64
CORES = 8
B = 512            # block edge
NB = N // B        # 16 block rows/cols
KT8 = D // 256     # 3 double-row contraction tiles
SLAB = N // CORES  # 1024 tm anchors per core
MT = SLAB // 128   # 8 tm row chunks
SW = [4, 4, 4, 2, 2, 1]   # strip widths (blocks) per core
NSTRIP = len(SW)
NBLK = sum(SW)     # 17 blocks per core
SOFF = [sum(SW[:s]) * B for s in range(NSTRIP)]   # column offsets in z8s
TCOLS = NBLK * B   # 8704 packed z8 columns per core
TEMP_INV = 10.0
EPS = 1e-8

FP8 = ml_dtypes.float8_e4m3
# True if the Act accum_out sums post-quantization (e5m2-rounded) values;
# determined empirically on hardware.
SELF_E5 = False

_CACHE = {}
LAST_RESULT = None


def _schedule():
    """Partition the 136 upper-triangle blocks into 8 cores x 6 same-row
    strips of widths [4,4,4,2,2,1]. Returns per-core strip lists of
    (i, [j...]) tuples. Deterministic, input-independent."""
    fours, twos, ones = [], [], []
    for i in range(NB):
        js = list(range(i, NB))
        L = len(js)
        o = L % 2
        rem = L - o
        t = (rem % 4) // 2
        f = (rem - 2 * t) // 4
        if i in (0, 4, 8, 12):   # rebalance 28/8/8 -> 24/16/8
            f -= 1
            t += 2
        pos = 0
        for _ in range(f):
            fours.append((i, js[pos:pos + 4])); pos += 4
        for _ in range(t):
            twos.append((i, js[pos:pos + 2])); pos += 2
        for _ in range(o):
            ones.append((i, js[pos:pos + 1])); pos += 1
        assert pos == L
    assert len(fours) == 24 and len(twos) == 16 and len(ones) == 8
    cores = []
    for c in range(CORES):
        cores.append([fours[3 * c], fours[3 * c + 1], fours[3 * c + 2],
                      twos[2 * c], twos[2 * c + 1], ones[c]])
    # coverage check: exactly the upper triangle, each block once
    seen = set()
    for strips in cores:
        for i, js in strips:
            for j in js:
                assert j >= i and (i, j) not in seen
                seen.add((i, j))
    assert len(seen) == 136
    return cores


SCHEDULE = _schedule()


def _build_nc():
    from concourse import bacc
    import concourse.mybir as mybir
    import concourse.tile as tile

    f8 = mybir.dt.float8e4
    f8e5 = mybir.dt.float8e5
    f32 = mybir.dt.float32
    bf16 = mybir.dt.bfloat16
    Exp = mybir.ActivationFunctionType.Exp
    DR = mybir.MatmulPerfMode.DoubleRow

    nc = bacc.Bacc(
        "TRN2", target_bir_lowering=False, debug=False, enable_asserts=False
    )
    z8s = nc.dram_tensor("z8s", [128, KT8, 2, TCOLS], f8, kind="ExternalInput").ap()
    a8s = nc.dram_tensor("a8s", [128, KT8, 2, NSTRIP, B], f8, kind="ExternalInput").ap()
    a8t = nc.dram_tensor("a8t", [128, KT8, 2, SLAB], f8, kind="ExternalInput").ap()
    w8 = nc.dram_tensor("w8", [128, KT8, 2, NOP], f8, kind="ExternalInput").ap()
    tm = nc.dram_tensor("tm", [128, MT, NOP], f32, kind="ExternalOutput").ap()
    pacc = nc.dram_tensor("pacc", [128, NSTRIP, 4], f32, kind="ExternalOutput").ap()
    eout = nc.dram_tensor("eout", [128, 4, TCOLS], f8e5, kind="ExternalOutput").ap()

    with tile.TileContext(nc) as tc:
        with (
            tc.tile_pool(name="zin", bufs=1) as zin,
            tc.tile_pool(name="epool", bufs=3) as epool,
            tc.tile_pool(name="singles", bufs=1) as singles,
        ):
            # ---- input DMAs, in consumption order ----
            w8_sb = singles.tile([128, KT8, 2, NOP], f8)
            nc.sync.dma_start(out=w8_sb, in_=w8)
            a8t_sb = singles.tile([128, KT8, 2, SLAB], f8)
            nc.sync.dma_start(
                out=a8t_sb.rearrange("p a b c -> p (a b c)"),
                in_=a8t.rearrange("p a b c -> p (a b c)"),
            )
            a8s_sb = {}
            z8_sb = {}

            def dma_strip(s):
                a8_t = zin.tile([128, KT8, 2, B], f8, name="a8_t",
                                tag=f"a8_{s}")
                nc.sync.dma_start(out=a8_t, in_=a8s[:, :, :, s, :])
                a8s_sb[s] = a8_t
                w = SW[s] * B
                for kk in range(KT8):
                    z8_t = zin.tile([128, 2, w], f8, name="z8_t",
                                    tag=f"z8_{s}_{kk}")
                    nc.sync.dma_start(
                        out=z8_t, in_=z8s[:, kk, :, SOFF[s]:SOFF[s] + w]
                    )
                    z8_sb[(s, kk)] = z8_t

            for s in range(NSTRIP):
                dma_strip(s)

            pacc_sb = singles.tile([128, NSTRIP, 4], f32)
            tm_sb = singles.tile([128, MT, NOP], f32)

            ps_pool = tc.alloc_tile_pool(name="ps", bufs=2, space="PSUM")

            # ---- class-segment sums: tm[:, m, c] = A_m @ W.T ----
            for m in range(MT):
                pst = ps_pool.tile([128, NOP], f32, name="ps_t", tag="ps_t")
                for kk in range(KT8):
                    nc.tensor.matmul(
                        pst,
                        a8t_sb[:, kk, :, m * 128:(m + 1) * 128],
                        w8_sb[:, kk, :, :],
                        start=(kk == 0),
                        stop=(kk == KT8 - 1),
                        perf_mode=DR,
                    )
                nc.vector.tensor_copy(tm_sb[:, m, :], pst)
            nc.sync.dma_start(out=tm, in_=tm_sb)

            # ---- upper-triangle strips ----
            for s in range(NSTRIP):
                w = SW[s] * B
                e_t = epool.tile([128, 4, w], f8e5, name="e_t", tag="e_t")
                for m in range(4):
                    ps_t = ps_pool.tile([128, w], f32, name="ps_t", tag="ps_t")
                    for kk in range(KT8):
                        lhsT = a8s_sb[s][:, kk, :, m * 128:(m + 1) * 128]
                        for jj in range(SW[s]):
                            nc.tensor.matmul(
                                ps_t[:, jj * B:(jj + 1) * B],
                                lhsT,
                                z8_sb[(s, kk)][:, :, jj * B:(jj + 1) * B],
                                start=(kk == 0),
                                stop=(kk == KT8 - 1),
                                perf_mode=DR,
                            )
                    nc.scalar.activation(
                        out=e_t[:, m, :],
                        in_=ps_t,
                        func=Exp,
                        scale=TEMP_INV,
                        accum_out=pacc_sb[:, s, m:m + 1],
                    )
                # export on the scalar HWDGE queue: runs parallel to the
                # input stream on the sync queue. Last strip split per
                # m-pair so the tail's final transfer is small.
                dst = eout[:, :, SOFF[s]:SOFF[s] + w]
                if s == NSTRIP - 1:
                    nc.scalar.dma_start(out=dst[:, 0:2, :], in_=e_t[:, 0:2, :])
                    nc.scalar.dma_start(out=dst[:, 2:4, :], in_=e_t[:, 2:4, :])
                else:
                    nc.scalar.dma_start(out=dst, in_=e_t)
            ps_pool.release()

            nc.sync.dma_start(out=pacc, in_=pacc_sb)

    nc.compile()
    return nc


def _get_nc():
    if "nc" not in _CACHE:
        _CACHE["nc"] = _build_nc()
    return _CACHE["nc"]


def _pack_dr(mat_t):
    """[D, cols] -> [128, KT8, 2, cols] with d = kk*256 + i*128 + p."""
    d, cols = mat_t.shape
    return np.ascontiguousarray(
        mat_t.reshape(KT8, 2, 128, cols).transpose(2, 0, 1, 3)
    )


def kernel(x, op_ids, n_op):
    global LAST_RESULT
    from concourse.bass_utils import run_bass_kernel_spmd

    x = np.asarray(x, dtype=np.float32).reshape(-1, D)
    op_ids = np.asarray(op_ids).reshape(-1).astype(np.int64)
    n_op_i = int(np.asarray(n_op))

    # ---- host prep: normalize, quantize, class sums, diagonal ----
    norms = np.sqrt((x.astype(np.float64) ** 2).sum(axis=1))
    norms = np.maximum(norms, EPS).astype(np.float32)
    z = x / norms[:, None]

    z8 = z.astype(FP8)
    z8f = z8.astype(np.float32)

    onehot = np.zeros((N, NOP), np.float32)
    onehot[np.arange(N), op_ids] = 1.0
    W8 = (onehot.T @ z8f).astype(FP8)

    z8_packed = _pack_dr(np.ascontiguousarray(z8.T))          # [128,3,2,N]
    w8_packed = _pack_dr(np.ascontiguousarray(W8.T.astype(FP8)))
    ssq = (z8f.astype(np.float64) ** 2).sum(axis=1)           # = sim[i, i]

    in_maps = []
    for c in range(CORES):
        strips = SCHEDULE[c]
        # z8s: strip blocks' columns concatenated in strip order
        zcols = np.concatenate(
            [z8_packed[:, :, :, j * B:(j + 1) * B] for i, js in strips
             for j in js],
            axis=3,
        )
        # a8s: strip stationaries (rows of strip's i)
        acols = np.stack(
            [z8_packed[:, :, :, i * B:(i + 1) * B] for i, js in strips],
            axis=3,
        )
        in_maps.append({
            "z8s": np.ascontiguousarray(zcols),
            "a8s": np.ascontiguousarray(acols),
            "a8t": np.ascontiguousarray(
                z8_packed[:, :, :, c * SLAB:(c + 1) * SLAB]
            ),
            "w8": w8_packed,
        })

    nc = _get_nc()
    res = run_bass_kernel_spmd(nc, in_maps, core_ids=list(range(CORES)))
    LAST_RESULT = res

    # ---- host post: stitch row + mirror sums, finish loss in f64 ----
    es = np.zeros(N, dtype=np.float64)
    tm_full = np.empty((N, NOP), dtype=np.float64)
    for c in range(CORES):
        strips = SCHEDULE[c]
        pacc_c = res.results[c]["pacc"].astype(np.float64)  # [128, 6, 4]
        eout_c = res.results[c]["eout"].astype(np.float64)  # [128, 4, TCOLS]
        ecs = eout_c.sum(axis=(0, 1))                       # per-column sums
        for s, (i, js) in enumerate(strips):
            for m in range(4):
                a0 = i * B + m * 128
                es[a0:a0 + 128] += pacc_c[:, s, m]
            for b, j in enumerate(js):
                if j != i:
                    o = SOFF[s] + b * B
                    es[j * B:(j + 1) * B] += ecs[o:o + B]
        tm_full[c * SLAB:(c + 1) * SLAB] = (
            res.results[c]["tm"].transpose(1, 0, 2).reshape(SLAB, NOP)
        )

    if SELF_E5:
        es -= np.exp(TEMP_INV * ssq).astype(np.float32).astype(
            ml_dtypes.float8_e5m2).astype(np.float64)
    else:
        es -= np.exp(TEMP_INV * ssq)      # remove self-term
    lse = np.log(es)
    pos_sum = TEMP_INV * (tm_full[np.arange(N), op_ids] - ssq)
    counts = np.bincount(op_ids, minlength=n_op_i).astype(np.float64)
    pos_cnt = counts[op_ids] - 1.0

    loss_i = np.where(pos_cnt > 0, -pos_sum / np.maximum(pos_cnt, 1.0) + lse, 0.0)
    cls_sum = np.bincount(op_ids, weights=loss_i, minlength=n_op_i)
    cls_loss = np.where(counts > 0, cls_sum / np.maximum(counts, 1.0), 0.0)
    return np.float32(cls_loss.mean())
